# revision 30
# baseline (speedup 1.0000x reference)
"""Trainium2 Bass kernel for nn_NetSpacing (weighted-average wirelength with
direction penalty; segment reductions over sorted pin->net runs).

Strategy (8 NeuronCores, SPMD):
  - Host: split pins at net boundaries into 8 shards (sharding hint); expand
    net_weights*net_mask to per-pin weights ("net arrays sharded to match").
  - Device (per core): pins in partition-major layout [128 rows x SW cols].
    Chunks of W columns are processed as overlapping windows
    [c0-1, c0+W+H) (V = W+H+1 cols); halo H >= max run length, so every run
    whose start lies in the core region is fully contained in the window.
    Per-net reductions are segmented scans along the free dim
    (tensor_tensor_scan), per-net values are broadcast back with reverse
    scans, and each net's contribution is evaluated at its run-end position,
    gated by a start-position validity test so each net counts exactly once.
  - Host: sum the 8x[128] partial outputs.
"""
import sys

for _p in ("/opt/trn_rl_repo",):
    if _p not in sys.path:
        sys.path.insert(0, _p)

from contextlib import ExitStack

import numpy as np

import concourse.bass as bass
import concourse.bacc as bacc
import concourse.tile as tile
from concourse import mybir
from concourse.bass_utils import run_bass_kernel_spmd

GAMMA = 4.0
C_THRESH = 0.5
NCORES = 8
NROWS = 128
NCHUNKS = 4

# tuning knobs (overridable before build_program)
KNOBS = {
    "bufs_work": 2,     # pw pool bufs
    "bufs_stream": 3,   # extra-buffered stream tags (0 = same as bufs_work)
    "dma_accum": True,  # use SWDGE accum DMAs for pure adds
    "min_streams": True,   # min-scans for low side (False: negate + max)
    "abs_validity": True,  # |q| <= thresh via ACT Abs (False: 2 compares)
    "custom_dve": True,    # fused custom DVE ops (sqsum / penalty / rangew)
}

# All activation funcs used here (Copy/Exp/Ln/Relu/Square/Abs) live in the
# single "natural_log_exp_and_others" table set; restricting table choice to
# it collapses ~49 LoadActFuncSet instructions (~63us of ACT time) to one.
from concourse import hw_specs as _hw_specs

_orig_gat = _hw_specs.get_activation_tables


def _gat_one_table(arch):
    # Keep every table and its position (act_func_set_id is positional into
    # act_info.json) but empty the others so the chooser always lands on
    # natural_log_exp_and_others.
    t = _orig_gat(arch)
    if "natural_log_exp_and_others" not in t:
        return t
    out = {}
    for k, v in t.items():
        out[k] = v if k == "natural_log_exp_and_others" else type(v)()
    return out


bacc.get_activation_tables = _gat_one_table


# ---- custom fused DVE ops ------------------------------------------------
from concourse import dve_ops as _dve_ops
from concourse.dve_spec import Spec as _Spec, Src0 as _S0, Src1 as _S1, \
    C0 as _C0, C1 as _C1, sq as _sq, relu as _relu
from concourse.dve_table_gen import dve_ver_for as _dve_ver_for
from concourse.dve_uop import DveOpSpec as _DveOpSpec
from concourse.dve_ops import get_dve_sub_opcode as _get_sub
from concourse.dve_spec import lower as _dve_lower


def _register_custom_op(name, spec):
    if name in _dve_ops._SUB_OPCODE_FOR_NAME:
        for op in _dve_ops.OPS:
            if op.name == name:
                return op
    row = _dve_ops._CUSTOM_DVE_ROW_BASE + len(_dve_ops.OPS)
    assert row < 0x20
    _dve_ops._SUB_OPCODE_FOR_NAME[name] = row
    shas = {}
    for ver in ("v3", "v4"):
        s = _DveOpSpec(
            name=name, opcode=row, uops=_dve_lower(spec, ver=ver),
            rd1_en=True,
        )
        shas[ver] = s.sha(ver)
    op = _dve_ops.DveOp(name, spec, subdim=False, uops_sha=shas)
    _dve_ops.OPS.append(op)
    _dve_ops.CUSTOM_DVE_SPECS[name] = spec
    return op


OP_SQSUM = _register_custom_op(
    "SQSUM_ANT",
    _Spec(
        body=_sq(_S0) + _sq(_S1),
        reference=lambda in0, in1, s0, s1, imm2: (
            in0 * in0 + in1 * in1
        ).astype(np.float32),
    ),
)
OP_PEN = _register_custom_op(
    "PEN_ANT",
    _Spec(
        body=_relu(_C0 - _S0 * _S1),
        reference=lambda in0, in1, s0, s1, imm2: np.maximum(
            s0 - in0 * in1, 0.0
        ).astype(np.float32),
    ),
)
OP_RANGEW = _register_custom_op(
    "RANGEW_ANT",
    _Spec(
        body=(_S0 < _C0) * (_C1 < _S0) * _S1,
        reference=lambda in0, in1, s0, s1, imm2: (
            (in0 < s0) & (s1 < in0)
        ).astype(np.float32)
        * in1,
    ),
)

F32 = mybir.dt.float32
I32 = mybir.dt.int32
OP = mybir.AluOpType
AF = mybir.ActivationFunctionType


def _rev(ap):
    """Reverse the free dim of a 2-D AP."""
    pairs = [list(x) for x in ap.ap]
    assert len(pairs) == 2, pairs
    step, cnt = pairs[1]
    return bass.AP(
        tensor=ap.tensor,
        offset=ap.offset + step * (cnt - 1),
        ap=[pairs[0], [-step, cnt]],
    )


def _win(dram_1d, col0, SW, V):
    """Window AP into the padded 1-D DRAM array: [128 rows x V cols], row p
    starting at element p*SW + col0."""
    return bass.AP(
        tensor=dram_1d.tensor,
        offset=dram_1d.offset + col0,
        ap=[[SW, NROWS], [1, V]],
    )


def build_program(SW, H, nchunks=None, repeat=1):
    """Build the SPMD bass program for per-row length SW, halo H.
    repeat>1 re-runs the whole compute (timing amplification only)."""
    nchunks = nchunks or NCHUNKS
    assert SW % nchunks == 0
    W = SW // nchunks
    V = W + H + 1
    PAD = NROWS * SW + H + 2
    dma_accum = KNOBS["dma_accum"]

    nc = bacc.Bacc("TRN2", target_bir_lowering=False, debug=False)
    d_seg = nc.dram_tensor("seg", [PAD], I32, kind="ExternalInput")
    d_x = nc.dram_tensor("x", [PAD], F32, kind="ExternalInput")
    d_y = nc.dram_tensor("y", [PAD], F32, kind="ExternalInput")
    d_px = nc.dram_tensor("px", [PAD], F32, kind="ExternalInput")
    d_py = nc.dram_tensor("py", [PAD], F32, kind="ExternalInput")
    d_w = nc.dram_tensor("w", [PAD], F32, kind="ExternalInput")
    d_out = nc.dram_tensor("out", [NROWS, 1], F32, kind="ExternalOutput")


    def _acc(dst, src):
        """dst += src on the DMA engines (SWDGE accumulate) or Pool."""
        if dma_accum:
            nc.gpsimd.dma_start(dst[:, :], src[:, :], accum_op=OP.add)
        else:
            nc.gpsimd.tensor_add(dst, dst, src)

    with tile.TileContext(nc) as tc, ExitStack() as ctx:
        consts = ctx.enter_context(tc.tile_pool(name="consts", bufs=1))
        pin = ctx.enter_context(tc.tile_pool(name="pin", bufs=2))
        pw = ctx.enter_context(
            tc.tile_pool(name="pw", bufs=KNOBS["bufs_work"])
        )
        ps = (
            ctx.enter_context(
                tc.tile_pool(name="ps", bufs=KNOBS["bufs_stream"])
            )
            if KNOBS["bufs_stream"]
            else pw
        )

        # constants
        iota2 = consts.tile([NROWS, V], F32)
        iota_i = consts.tile([NROWS, V], I32)
        nc.gpsimd.iota(iota_i, pattern=[[1, V]], base=0, channel_multiplier=0)
        # iota2 = iota - (W-1)/2   (pre-shifted for the |.| validity test)
        nc.vector.tensor_copy(iota2, iota_i)
        nc.vector.tensor_scalar_sub(iota2, iota2, (W - 1) / 2.0)
        ones = consts.tile([NROWS, V], F32)
        nc.vector.memset(ones, 1.0)
        b_zero = consts.tile([NROWS, 1], F32)
        nc.vector.memset(b_zero, 0.0)
        b_e30 = consts.tile([NROWS, 1], F32)
        nc.vector.memset(b_e30, 1e-30)
        b_e16 = consts.tile([NROWS, 1], F32)
        nc.vector.memset(b_e16, 1e-16)
        b_half = consts.tile([NROWS, 1], F32)
        nc.vector.memset(b_half, C_THRESH)
        acc_total = consts.tile([NROWS, 1], F32)
        nc.vector.memset(acc_total, 0.0)

        for rep in range(repeat):
          for j in range(nchunks):
            c0 = j * W
            # ---- loads ----
            seg_t = pin.tile([NROWS, V], I32, tag="seg")
            nc.sync.dma_start(seg_t, _win(d_seg[:], c0, SW, V))
            x_t = pin.tile([NROWS, V], F32, tag="x")
            nc.sync.dma_start(x_t, _win(d_x[:], c0, SW, V))
            y_t = pin.tile([NROWS, V], F32, tag="y")
            nc.sync.dma_start(y_t, _win(d_y[:], c0, SW, V))
            px_t = pin.tile([NROWS, V], F32, tag="px")
            nc.sync.dma_start(px_t, _win(d_px[:], c0, SW, V))
            py_t = pin.tile([NROWS, V], F32, tag="py")
            nc.sync.dma_start(py_t, _win(d_py[:], c0, SW, V))
            w_t = pin.tile([NROWS, V], F32, tag="w")
            nc.sync.dma_start(w_t, _win(d_w[:], c0, SW, V))

            # ---- masks ----
            M = pw.tile([NROWS, V + 1], F32, tag="M")
            nc.vector.memset(M[:, 0:1], 0.0)
            nc.vector.memset(M[:, V : V + 1], 0.0)
            nc.vector.tensor_tensor(
                M[:, 1:V], seg_t[:, 1:V], seg_t[:, 0 : V - 1], OP.is_equal
            )
            Mneg = pw.tile([NROWS, V + 1], F32, tag="Mneg")
            nc.scalar.activation(Mneg, M, AF.Copy, bias=-1e30, scale=1e30)
            if KNOBS["min_streams"]:
                Mpos = pw.tile([NROWS, V + 1], F32, tag="Mpos")
                nc.scalar.activation(Mpos, M, AF.Copy, bias=1e30, scale=-1e30)
            else:
                Mpos = None
            islast = pw.tile([NROWS, V], F32, tag="islast")
            nc.scalar.activation(
                islast, M[:, 1 : V + 1], AF.Copy, bias=1.0, scale=-1.0
            )

            # ---- four streams: (x,max), (y,max), (x,min), (y,min) ----
            # wa_x+wa_y = (Mx-mx)+(My-my) + sum_streams phi-sums, with
            # phi = d*exp(d/g), d = v - runmax (max streams) or runmin - v.
            acc_r = pw.tile([NROWS, V], F32, tag="acc_r")
            acc_vb = pw.tile([NROWS, V], F32, tag="acc_vb")

            for si in range(4):
                is_min = si >= 2 and KNOBS["min_streams"]
                if si < 2 or KNOBS["min_streams"]:
                    v_t = x_t if si % 2 == 0 else y_t
                else:
                    nv = ps.tile([NROWS, V], F32, tag="nv")
                    nc.scalar.activation(
                        nv, x_t if si == 2 else y_t, AF.Copy, bias=0.0,
                        scale=-1.0,
                    )
                    v_t = nv
                Mb = Mpos if is_min else Mneg
                init = 3e38 if is_min else -3e38
                opm = OP.min if is_min else OP.max
                mx_s = ps.tile([NROWS, V], F32, tag="mx_s")
                nc.vector.tensor_tensor_scan(
                    mx_s, Mb[:, 0:V], v_t, init, OP.add, opm
                )
                # run max/min broadcast; first stream writes acc_vb directly
                vb = acc_vb if si == 0 else ps.tile([NROWS, V], F32, tag="vb")
                nc.vector.tensor_tensor_scan(
                    _rev(vb[:, 0:V]),
                    _rev(Mb[:, 1 : V + 1]),
                    _rev(mx_s[:, 0:V]),
                    init,
                    OP.add,
                    opm,
                )
                d_t = ps.tile([NROWS, V], F32, tag="d")
                if is_min:
                    nc.vector.tensor_sub(d_t, vb, v_t)
                    # acc_vb -= runmin
                    nc.gpsimd.tensor_sub(acc_vb, acc_vb, vb)
                else:
                    nc.vector.tensor_sub(d_t, v_t, vb)
                    if si >= 1:
                        # acc_vb += run max (of v or -v)
                        _acc(acc_vb, vb)
                e_t = ps.tile([NROWS, V], F32, tag="e")
                nc.scalar.activation(
                    e_t, d_t, AF.Exp, bias=b_zero, scale=1.0 / GAMMA
                )
                # p = d*e (into d)
                nc.gpsimd.tensor_mul(d_t, d_t, e_t)
                se_s = ps.tile([NROWS, V], F32, tag="se_s")
                nc.vector.tensor_tensor_scan(
                    se_s, M[:, 0:V], e_t, 0.0, OP.mult, OP.add
                )
                sp_s = ps.tile([NROWS, V], F32, tag="sp_s")
                nc.vector.tensor_tensor_scan(
                    sp_s, M[:, 0:V], d_t, 0.0, OP.mult, OP.add
                )
                # rse = 1/se  (in place over se_s)
                nc.scalar.activation(se_s, se_s, AF.Ln, bias=b_e30)
                nc.scalar.activation(se_s, se_s, AF.Exp, bias=b_zero, scale=-1.0)
                m1_eng = nc.gpsimd if si >= 2 else nc.vector
                if si == 0:
                    # m1 = sp*rse written straight into acc_r
                    m1_eng.tensor_mul(acc_r, sp_s, se_s)
                else:
                    m1_eng.tensor_mul(sp_s, sp_s, se_s)
                    # acc_r += m1
                    _acc(acc_r, sp_s)

            # ---- count / centroid / penalty ----
            c_s = pw.tile([NROWS, V], F32, tag="c_s")
            nc.vector.tensor_tensor_scan(
                c_s, M[:, 0:V], ones, 0.0, OP.mult, OP.add
            )
            # rcl = islast/count (islast folded in; only run-end values are
            # ever consumed downstream)
            rcl = pw.tile([NROWS, V], F32, tag="rcl")
            nc.scalar.activation(rcl, c_s, AF.Ln, bias=b_zero)
            nc.scalar.activation(rcl, rcl, AF.Exp, bias=b_zero, scale=-1.0)
            nc.gpsimd.tensor_mul(rcl, rcl, islast)

            sx_s = ps.tile([NROWS, V], F32, tag="mx_s")
            nc.vector.tensor_tensor_scan(
                sx_s, M[:, 0:V], x_t, 0.0, OP.mult, OP.add
            )
            sy_s = ps.tile([NROWS, V], F32, tag="vb")
            nc.vector.tensor_tensor_scan(
                sy_s, M[:, 0:V], y_t, 0.0, OP.mult, OP.add
            )
            # cxl = sx*rcl (rcl already includes islast)
            nc.gpsimd.tensor_mul(sx_s, sx_s, rcl)
            nc.gpsimd.tensor_mul(sy_s, sy_s, rcl)
            CX = ps.tile([NROWS, V], F32, tag="d")
            nc.vector.tensor_tensor_scan(
                _rev(CX[:, 0:V]),
                _rev(M[:, 1 : V + 1]),
                _rev(sx_s[:, 0:V]),
                0.0,
                OP.mult,
                OP.add,
            )
            CY = ps.tile([NROWS, V], F32, tag="e")
            nc.vector.tensor_tensor_scan(
                _rev(CY[:, 0:V]),
                _rev(M[:, 1 : V + 1]),
                _rev(sy_s[:, 0:V]),
                0.0,
                OP.mult,
                OP.add,
            )
            # dxp = CX - x (in place), dyp = CY - y (in place)
            nc.vector.tensor_sub(CX, CX, x_t)
            nc.vector.tensor_sub(CY, CY, y_t)
            dx2 = ps.tile([NROWS, V], F32, tag="se_s")
            if KNOBS["custom_dve"]:
                # d2 = dxp^2 + dyp^2 in one fused DVE op
                nc.vector._custom_dve(OP_SQSUM, out=dx2, in0=CX, in1=CY)
            else:
                nc.scalar.activation(dx2, CX, AF.Square, bias=b_zero)
                dy2 = ps.tile([NROWS, V], F32, tag="sp_s")
                nc.scalar.activation(dy2, CY, AF.Square, bias=b_zero)
                _acc(dx2, dy2)
            # rdn = (d2 + 1e-16)^-0.5 in place
            nc.scalar.activation(dx2, dx2, AF.Ln, bias=b_e16)
            nc.scalar.activation(dx2, dx2, AF.Exp, bias=b_zero, scale=-0.5)
            # n1 = dxp*px (into CX), n2 = dyp*py (into CY)
            nc.gpsimd.tensor_mul(CX, CX, px_t)
            nc.gpsimd.tensor_mul(CY, CY, py_t)
            # num = n1+n2 (DMA accumulate into CX)
            _acc(CX, CY)
            if KNOBS["custom_dve"]:
                # pen = relu(0.5 - num*rdn) in one fused DVE op (into CY)
                nc.vector._custom_dve(
                    OP_PEN, out=CY, in0=CX, in1=dx2, s0=C_THRESH
                )
            else:
                nc.vector.tensor_mul(CX, CX, dx2)
                nc.scalar.activation(CY, CX, AF.Relu, bias=b_half, scale=-1.0)
            pen_s = ps.tile([NROWS, V], F32, tag="sp_s")
            nc.vector.tensor_tensor_scan(
                pen_s, M[:, 0:V], CY, 0.0, OP.mult, OP.add
            )
            # wt = pen_s*rcl (in place)
            nc.gpsimd.tensor_mul(pen_s, pen_s, rcl)

            # ---- validity: q = iota - c_s; valid iff 0 <= q <= W-1,
            # i.e. |q - (W-1)/2| <= (W-1)/2 (iota2 pre-shifted) ----
            q = ps.tile([NROWS, V], F32, tag="e")
            nc.vector.scalar_tensor_tensor(
                q, c_s, -1.0, iota2, OP.mult, OP.add
            )
            v1 = ps.tile([NROWS, V], F32, tag="mx_s")
            if KNOBS["custom_dve"]:
                # wi = w*islast, then wvl = (|q| in range) * wi fused
                nc.gpsimd.tensor_mul(v1, w_t, islast)
                hw = (W - 1) / 2.0 + 0.5
                nc.vector._custom_dve(
                    OP_RANGEW, out=v1, in0=q, in1=v1, s0=hw, s1=-hw
                )
            elif KNOBS["abs_validity"]:
                nc.scalar.activation(q, q, AF.Abs, bias=b_zero)
                nc.gpsimd.tensor_scalar(
                    v1, q, (W - 1) / 2.0, None, OP.is_le
                )
                nc.gpsimd.tensor_mul(v1, v1, w_t)
                nc.gpsimd.tensor_mul(v1, v1, islast)
            else:
                hw = (W - 1) / 2.0
                v2 = ps.tile([NROWS, V], F32, tag="vb")
                nc.gpsimd.tensor_scalar(v1, q, -hw, None, OP.is_ge)
                nc.gpsimd.tensor_scalar(v2, q, hw, None, OP.is_le)
                nc.gpsimd.tensor_mul(v1, v1, v2)
                nc.gpsimd.tensor_mul(v1, v1, w_t)
                nc.gpsimd.tensor_mul(v1, v1, islast)

            # ---- final ----
            # wlx = acc_r + acc_vb (DMA accumulate); wl = relu (in place)
            _acc(acc_r, acc_vb)
            nc.scalar.activation(acc_r, acc_r, AF.Relu, bias=b_zero)
            # f1 = (wt + 1) * wl (into pen_s)
            nc.vector.scalar_tensor_tensor(
                pen_s, pen_s, 1.0, acc_r, OP.add, OP.mult
            )
            # f2 = f1 * wvl, accumulate row-sums
            f2 = pw.tile([NROWS, V], F32, tag="acc_vb")
            acc_j = pw.tile([NROWS, 1], F32, tag="acc_j")
            nc.vector.scalar_tensor_tensor(
                f2, pen_s, 0.0, v1, OP.add, OP.mult, accum_out=acc_j
            )
            nc.vector.tensor_add(acc_total, acc_total, acc_j)

        nc.sync.dma_start(d_out[:, :], acc_total)
    nc.compile()
    return nc


_PROG_CACHE = {}


def _get_program(SW, H):
    key = (SW, H)
    if key not in _PROG_CACHE:
        _PROG_CACHE[key] = build_program(SW, H)
    return _PROG_CACHE[key]


def prepare(pos, pin_dir_x, pin_dir_y, net_weights, pin2net_map, net_mask,
            pin_mask=None):
    """Host-side sharding/padding. Returns (nc, in_maps, meta)."""
    P = int(pin_dir_x.shape[0])
    x = np.ascontiguousarray(np.asarray(pos[:P], dtype=np.float32))
    y = np.ascontiguousarray(np.asarray(pos[P:], dtype=np.float32))
    seg = np.asarray(pin2net_map, dtype=np.int32)
    px = np.asarray(pin_dir_x, dtype=np.float32)
    py = np.asarray(pin_dir_y, dtype=np.float32)
    wm = np.asarray(net_weights, dtype=np.float32) * np.asarray(
        net_mask
    ).astype(np.float32)
    w_pin = wm[seg]

    # max run length -> halo (needs H >= Lmax - 1; +1 margin, mult of 8)
    counts = np.bincount(seg)
    Lmax = int(counts.max()) if counts.size else 1
    H = max(24, -(-(Lmax + 1) // 8) * 8)

    # net-boundary shard splits
    bounds = [0]
    for c in range(1, NCORES):
        tgt = c * P // NCORES
        bounds.append(int(np.searchsorted(seg, seg[tgt], side="left")))
    bounds.append(P)
    maxL = max(bounds[i + 1] - bounds[i] for i in range(NCORES))
    SW = -(-maxL // NROWS)
    SW = -(-SW // 32) * 32  # multiple of 32 (W = SW/4 multiple of 8)
    PAD = NROWS * SW + H + 2

    in_maps = []
    for c in range(NCORES):
        lo, hi = bounds[c], bounds[c + 1]
        L = hi - lo

        def padarr(a, fill, dtype):
            out = np.full(PAD, fill, dtype)
            out[1 : 1 + L] = a[lo:hi]
            return out

        in_maps.append(
            {
                "seg": padarr(seg, -1, np.int32),
                "x": padarr(x, 0.0, np.float32),
                "y": padarr(y, 0.0, np.float32),
                "px": padarr(px, 0.0, np.float32),
                "py": padarr(py, 0.0, np.float32),
                "w": padarr(w_pin, 0.0, np.float32),
            }
        )
    nc = _get_program(SW, H)
    return nc, in_maps, {"SW": SW, "H": H, "PAD": PAD}


def kernel(**inputs):
    nc, in_maps, _ = prepare(**inputs)
    res = run_bass_kernel_spmd(nc, in_maps, list(range(NCORES)))
    total = np.float64(0.0)
    for r in res.results:
        total += np.asarray(r["out"], dtype=np.float64).sum()
    return np.float32(total)


if __name__ == "__main__":
    # tiny self-check with synthetic data
    rng = np.random.default_rng(0)
    Np, Nn = 1 << 14, 1 << 11
    seg = np.sort(rng.integers(0, Nn, Np)).astype(np.int32)
    inputs = dict(
        pos=rng.normal(size=2 * Np).astype(np.float32) * 100,
        pin_dir_x=rng.normal(size=Np).astype(np.float32),
        pin_dir_y=rng.normal(size=Np).astype(np.float32),
        net_weights=rng.random(Nn).astype(np.float32),
        pin2net_map=seg,
        net_mask=np.ones(Nn, bool),
        pin_mask=np.zeros(Np, bool),
    )
    print("result:", kernel(**inputs))


# revision 35
# speedup vs baseline: 2.1358x; 2.1358x over previous
"""Trainium2 Bass kernel for nn_NetSpacing (weighted-average wirelength with
direction penalty; segment reductions over sorted pin->net runs).

Strategy (8 NeuronCores, SPMD):
  - Host: split pins at net boundaries into 8 shards (sharding hint); expand
    net_weights*net_mask to per-pin weights ("net arrays sharded to match").
  - Device (per core): pins in partition-major layout [128 rows x SW cols].
    Chunks of W columns are processed as overlapping windows
    [c0-1, c0+W+H) (V = W+H+1 cols); halo H >= max run length, so every run
    whose start lies in the core region is fully contained in the window.
    Per-net reductions are segmented scans along the free dim
    (tensor_tensor_scan), per-net values are broadcast back with reverse
    scans, and each net's contribution is evaluated at its run-end position,
    gated by a start-position validity test so each net counts exactly once.
  - Host: sum the 8x[128] partial outputs.
"""
import sys

for _p in ("/opt/trn_rl_repo",):
    if _p not in sys.path:
        sys.path.insert(0, _p)

from contextlib import ExitStack

import numpy as np

import concourse.bass as bass
import concourse.bacc as bacc
import concourse.tile as tile
from concourse import mybir
from concourse.bass_utils import run_bass_kernel_spmd

GAMMA = 4.0
C_THRESH = 0.5
NCORES = 8
NROWS = 128
NCHUNKS = 4

# tuning knobs (overridable before build_program)
KNOBS = {
    "bufs_work": 2,     # pw pool bufs
    "bufs_stream": 3,   # extra-buffered stream tags (0 = same as bufs_work)
    "dma_accum": True,  # use SWDGE accum DMAs for pure adds
    "min_streams": True,   # min-scans for low side (False: negate + max)
    "abs_validity": True,  # |q| <= thresh via ACT Abs (False: 2 compares)
    "custom_dve": True,    # fused custom DVE ops (sqsum / penalty / rangew)
    "bf16_streams": False,  # bf16 e/p arrays + se/sp scans (fp32 scan state)
}

# All activation funcs used here (Copy/Exp/Ln/Relu/Square/Abs) live in the
# single "natural_log_exp_and_others" table set; restricting table choice to
# it collapses ~49 LoadActFuncSet instructions (~63us of ACT time) to one.
from concourse import hw_specs as _hw_specs

_orig_gat = _hw_specs.get_activation_tables


def _gat_one_table(arch):
    # Keep every table and its position (act_func_set_id is positional into
    # act_info.json) but empty the others so the chooser always lands on
    # natural_log_exp_and_others.
    t = _orig_gat(arch)
    if "natural_log_exp_and_others" not in t:
        return t
    out = {}
    for k, v in t.items():
        out[k] = v if k == "natural_log_exp_and_others" else type(v)()
    return out


bacc.get_activation_tables = _gat_one_table


# ---- custom fused DVE ops ------------------------------------------------
from concourse import dve_ops as _dve_ops
from concourse.dve_spec import Spec as _Spec, Src0 as _S0, Src1 as _S1, \
    C0 as _C0, C1 as _C1, sq as _sq, relu as _relu
from concourse.dve_table_gen import dve_ver_for as _dve_ver_for
from concourse.dve_uop import DveOpSpec as _DveOpSpec
from concourse.dve_ops import get_dve_sub_opcode as _get_sub
from concourse.dve_spec import lower as _dve_lower


def _register_custom_op(name, spec):
    if name in _dve_ops._SUB_OPCODE_FOR_NAME:
        for op in _dve_ops.OPS:
            if op.name == name:
                return op
    row = _dve_ops._CUSTOM_DVE_ROW_BASE + len(_dve_ops.OPS)
    assert row < 0x20
    _dve_ops._SUB_OPCODE_FOR_NAME[name] = row
    shas = {}
    for ver in ("v3", "v4"):
        s = _DveOpSpec(
            name=name, opcode=row, uops=_dve_lower(spec, ver=ver),
            rd1_en=True,
        )
        shas[ver] = s.sha(ver)
    op = _dve_ops.DveOp(name, spec, subdim=False, uops_sha=shas)
    _dve_ops.OPS.append(op)
    _dve_ops.CUSTOM_DVE_SPECS[name] = spec
    return op


OP_SQSUM = _register_custom_op(
    "SQSUM_ANT",
    _Spec(
        body=_sq(_S0) + _sq(_S1),
        reference=lambda in0, in1, s0, s1, imm2: (
            in0 * in0 + in1 * in1
        ).astype(np.float32),
    ),
)
OP_PEN = _register_custom_op(
    "PEN_ANT",
    _Spec(
        body=_relu(_C0 - _S0 * _S1),
        reference=lambda in0, in1, s0, s1, imm2: np.maximum(
            s0 - in0 * in1, 0.0
        ).astype(np.float32),
    ),
)
OP_RANGEW = _register_custom_op(
    "RANGEW_ANT",
    _Spec(
        body=(_S0 < _C0) * (_C1 < _S0) * _S1,
        reference=lambda in0, in1, s0, s1, imm2: (
            (in0 < s0) & (s1 < in0)
        ).astype(np.float32)
        * in1,
    ),
)

F32 = mybir.dt.float32
BF16 = mybir.dt.bfloat16
I32 = mybir.dt.int32
OP = mybir.AluOpType
AF = mybir.ActivationFunctionType


def _rev(ap):
    """Reverse the free dim of a 2-D AP."""
    pairs = [list(x) for x in ap.ap]
    assert len(pairs) == 2, pairs
    step, cnt = pairs[1]
    return bass.AP(
        tensor=ap.tensor,
        offset=ap.offset + step * (cnt - 1),
        ap=[pairs[0], [-step, cnt]],
    )


def _win(dram_1d, col0, SW, V):
    """Window AP into the padded 1-D DRAM array: [128 rows x V cols], row p
    starting at element p*SW + col0."""
    return bass.AP(
        tensor=dram_1d.tensor,
        offset=dram_1d.offset + col0,
        ap=[[SW, NROWS], [1, V]],
    )


def build_program(SW, H, nchunks=None, repeat=1):
    """Build the SPMD bass program for per-row length SW, halo H.
    repeat>1 re-runs the whole compute (timing amplification only)."""
    nchunks = nchunks or NCHUNKS
    assert SW % nchunks == 0
    W = SW // nchunks
    V = W + H + 1
    PAD = NROWS * SW + H + 2
    dma_accum = KNOBS["dma_accum"]

    nc = bacc.Bacc("TRN2", target_bir_lowering=False, debug=False)
    d_seg = nc.dram_tensor("seg", [PAD], I32, kind="ExternalInput")
    d_x = nc.dram_tensor("x", [PAD], F32, kind="ExternalInput")
    d_y = nc.dram_tensor("y", [PAD], F32, kind="ExternalInput")
    d_px = nc.dram_tensor("px", [PAD], F32, kind="ExternalInput")
    d_py = nc.dram_tensor("py", [PAD], F32, kind="ExternalInput")
    d_w = nc.dram_tensor("w", [PAD], F32, kind="ExternalInput")
    d_out = nc.dram_tensor("out", [NROWS, 1], F32, kind="ExternalOutput")


    def _acc(dst, src):
        """dst += src on the DMA engines (SWDGE accumulate) or Pool."""
        if dma_accum:
            nc.gpsimd.dma_start(dst[:, :], src[:, :], accum_op=OP.add)
        else:
            nc.gpsimd.tensor_add(dst, dst, src)

    with tile.TileContext(nc) as tc, ExitStack() as ctx:
        consts = ctx.enter_context(tc.tile_pool(name="consts", bufs=1))
        pin = ctx.enter_context(tc.tile_pool(name="pin", bufs=2))
        pw = ctx.enter_context(
            tc.tile_pool(name="pw", bufs=KNOBS["bufs_work"])
        )
        ps = (
            ctx.enter_context(
                tc.tile_pool(name="ps", bufs=KNOBS["bufs_stream"])
            )
            if KNOBS["bufs_stream"]
            else pw
        )

        # constants
        iota2 = consts.tile([NROWS, V], F32)
        iota_i = consts.tile([NROWS, V], I32)
        nc.gpsimd.iota(iota_i, pattern=[[1, V]], base=0, channel_multiplier=0)
        # iota2 = iota - (W-1)/2   (pre-shifted for the |.| validity test)
        nc.vector.tensor_copy(iota2, iota_i)
        nc.vector.tensor_scalar_sub(iota2, iota2, (W - 1) / 2.0 + 1.0)
        b_zero = consts.tile([NROWS, 1], F32)
        nc.vector.memset(b_zero, 0.0)
        b_e30 = consts.tile([NROWS, 1], F32)
        nc.vector.memset(b_e30, 1e-30)
        b_e16 = consts.tile([NROWS, 1], F32)
        nc.vector.memset(b_e16, 1e-16)
        b_half = consts.tile([NROWS, 1], F32)
        nc.vector.memset(b_half, C_THRESH)
        b_one = consts.tile([NROWS, 1], F32)
        nc.vector.memset(b_one, 1.0)
        acc_total = consts.tile([NROWS, 1], F32)
        nc.vector.memset(acc_total, 0.0)

        for rep in range(repeat):
          for j in range(nchunks):
            c0 = j * W
            # ---- loads ----
            seg_t = pin.tile([NROWS, V], I32, tag="seg")
            nc.sync.dma_start(seg_t, _win(d_seg[:], c0, SW, V))
            x_t = pin.tile([NROWS, V], F32, tag="x")
            nc.sync.dma_start(x_t, _win(d_x[:], c0, SW, V))
            y_t = pin.tile([NROWS, V], F32, tag="y")
            nc.sync.dma_start(y_t, _win(d_y[:], c0, SW, V))
            px_t = pin.tile([NROWS, V], F32, tag="px")
            nc.sync.dma_start(px_t, _win(d_px[:], c0, SW, V))
            py_t = pin.tile([NROWS, V], F32, tag="py")
            nc.sync.dma_start(py_t, _win(d_py[:], c0, SW, V))
            w_t = pin.tile([NROWS, V], F32, tag="w")
            nc.sync.dma_start(w_t, _win(d_w[:], c0, SW, V))

            # ---- masks ----
            M = pw.tile([NROWS, V + 1], F32, tag="M")
            nc.vector.memset(M[:, 0:1], 0.0)
            nc.vector.memset(M[:, V : V + 1], 0.0)
            nc.vector.tensor_tensor(
                M[:, 1:V], seg_t[:, 1:V], seg_t[:, 0 : V - 1], OP.is_equal
            )
            Mneg = pw.tile([NROWS, V + 1], F32, tag="Mneg")
            nc.scalar.activation(Mneg, M, AF.Copy, bias=-1e30, scale=1e30)
            if KNOBS["min_streams"]:
                Mpos = pw.tile([NROWS, V + 1], F32, tag="Mpos")
                nc.scalar.activation(Mpos, M, AF.Copy, bias=1e30, scale=-1e30)
            else:
                Mpos = None
            islast = pw.tile([NROWS, V], F32, tag="islast")
            nc.scalar.activation(
                islast, M[:, 1 : V + 1], AF.Copy, bias=1.0, scale=-1.0
            )
            if KNOBS["bf16_streams"]:
                M16 = pw.tile([NROWS, V], BF16, tag="M16")
                nc.scalar.activation(M16, M[:, 0:V], AF.Copy, bias=0.0)
            else:
                M16 = None

            # ---- four streams: (x,max), (y,max), (x,min), (y,min) ----
            # wa_x+wa_y = (Mx-mx)+(My-my) + sum_streams phi-sums, with
            # phi = d*exp(d/g), d = v - runmax (max streams) or runmin - v.
            acc_r = pw.tile([NROWS, V], F32, tag="acc_r")
            acc_vb = pw.tile([NROWS, V], F32, tag="acc_vb")

            for si in range(4):
                is_min = si >= 2 and KNOBS["min_streams"]
                if si < 2 or KNOBS["min_streams"]:
                    v_t = x_t if si % 2 == 0 else y_t
                else:
                    nv = ps.tile([NROWS, V], F32, tag="nv")
                    nc.scalar.activation(
                        nv, x_t if si == 2 else y_t, AF.Copy, bias=0.0,
                        scale=-1.0,
                    )
                    v_t = nv
                Mb = Mpos if is_min else Mneg
                init = 3e38 if is_min else -3e38
                opm = OP.min if is_min else OP.max
                mx_s = ps.tile([NROWS, V], F32, tag="mx_s")
                nc.vector.tensor_tensor_scan(
                    mx_s, Mb[:, 0:V], v_t, init, OP.add, opm
                )
                # run max/min broadcast; first stream writes acc_vb directly
                vb = acc_vb if si == 0 else ps.tile([NROWS, V], F32, tag="vb")
                nc.vector.tensor_tensor_scan(
                    _rev(vb[:, 0:V]),
                    _rev(Mb[:, 1 : V + 1]),
                    _rev(mx_s[:, 0:V]),
                    init,
                    OP.add,
                    opm,
                )
                sdt = BF16 if KNOBS["bf16_streams"] else F32
                Msc = M16 if KNOBS["bf16_streams"] else M[:, 0:V]
                d_t = ps.tile([NROWS, V], sdt, tag="d")
                if is_min:
                    nc.vector.tensor_sub(d_t, vb, v_t)
                    # acc_vb -= runmin
                    nc.gpsimd.tensor_sub(acc_vb, acc_vb, vb)
                else:
                    nc.vector.tensor_sub(d_t, v_t, vb)
                    if si >= 1:
                        # acc_vb += run max (of v or -v)
                        _acc(acc_vb, vb)
                e_t = ps.tile([NROWS, V], sdt, tag="e")
                nc.scalar.activation(
                    e_t, d_t, AF.Exp, bias=b_zero, scale=1.0 / GAMMA
                )
                # p = d*e (into d)
                nc.gpsimd.tensor_mul(d_t, d_t, e_t)
                se_s = ps.tile([NROWS, V], sdt, tag="se_s")
                nc.vector.tensor_tensor_scan(
                    se_s, Msc, e_t, 0.0, OP.mult, OP.add
                )
                sp_s = ps.tile([NROWS, V], sdt, tag="sp_s")
                nc.vector.tensor_tensor_scan(
                    sp_s, Msc, d_t, 0.0, OP.mult, OP.add
                )
                # rse = 1/se  (in place over se_s)
                nc.scalar.activation(se_s, se_s, AF.Ln, bias=b_e30)
                nc.scalar.activation(se_s, se_s, AF.Exp, bias=b_zero, scale=-1.0)
                if si == 0:
                    # m1 = sp*rse written straight into acc_r
                    nc.vector.tensor_mul(acc_r, sp_s, se_s)
                else:
                    nc.vector.tensor_mul(sp_s, sp_s, se_s)
                    # acc_r += m1
                    _acc(acc_r, sp_s)

            # ---- count / centroid / penalty ----
            # c_s = (count-1): data1 = M (0 at run starts, 1 inside)
            c_s = pw.tile([NROWS, V], F32, tag="c_s")
            nc.vector.tensor_tensor_scan(
                c_s, M[:, 0:V], M[:, 0:V], 0.0, OP.mult, OP.add
            )
            # rcl = islast/count (islast folded in; only run-end values are
            # ever consumed downstream)
            rcl = pw.tile([NROWS, V], F32, tag="rcl")
            nc.scalar.activation(rcl, c_s, AF.Ln, bias=b_one)
            nc.scalar.activation(rcl, rcl, AF.Exp, bias=b_zero, scale=-1.0)
            nc.gpsimd.tensor_mul(rcl, rcl, islast)

            sx_s = ps.tile([NROWS, V], F32, tag="mx_s")
            nc.vector.tensor_tensor_scan(
                sx_s, M[:, 0:V], x_t, 0.0, OP.mult, OP.add
            )
            sy_s = ps.tile([NROWS, V], F32, tag="vb")
            nc.vector.tensor_tensor_scan(
                sy_s, M[:, 0:V], y_t, 0.0, OP.mult, OP.add
            )
            # cxl = sx*rcl (rcl already includes islast)
            nc.gpsimd.tensor_mul(sx_s, sx_s, rcl)
            nc.gpsimd.tensor_mul(sy_s, sy_s, rcl)
            CX = ps.tile([NROWS, V], F32, tag="d")
            nc.vector.tensor_tensor_scan(
                _rev(CX[:, 0:V]),
                _rev(M[:, 1 : V + 1]),
                _rev(sx_s[:, 0:V]),
                0.0,
                OP.mult,
                OP.add,
            )
            CY = ps.tile([NROWS, V], F32, tag="e")
            nc.vector.tensor_tensor_scan(
                _rev(CY[:, 0:V]),
                _rev(M[:, 1 : V + 1]),
                _rev(sy_s[:, 0:V]),
                0.0,
                OP.mult,
                OP.add,
            )
            # dxp = CX - x (in place), dyp = CY - y (in place)
            nc.vector.tensor_sub(CX, CX, x_t)
            nc.vector.tensor_sub(CY, CY, y_t)
            dx2 = ps.tile([NROWS, V], F32, tag="se_s")
            if KNOBS["custom_dve"]:
                # d2 = dxp^2 + dyp^2 in one fused DVE op
                nc.vector._custom_dve(OP_SQSUM, out=dx2, in0=CX, in1=CY)
            else:
                nc.scalar.activation(dx2, CX, AF.Square, bias=b_zero)
                dy2 = ps.tile([NROWS, V], F32, tag="sp_s")
                nc.scalar.activation(dy2, CY, AF.Square, bias=b_zero)
                _acc(dx2, dy2)
            # rdn = (d2 + 1e-16)^-0.5 in place
            nc.scalar.activation(dx2, dx2, AF.Ln, bias=b_e16)
            nc.scalar.activation(dx2, dx2, AF.Exp, bias=b_zero, scale=-0.5)
            # n1 = dxp*px (into CX), n2 = dyp*py (into CY)
            nc.gpsimd.tensor_mul(CX, CX, px_t)
            nc.gpsimd.tensor_mul(CY, CY, py_t)
            # num = n1+n2 (DMA accumulate into CX)
            _acc(CX, CY)
            if KNOBS["custom_dve"]:
                # pen = relu(0.5 - num*rdn) in one fused DVE op (into CY)
                nc.vector._custom_dve(
                    OP_PEN, out=CY, in0=CX, in1=dx2, s0=C_THRESH
                )
            else:
                nc.vector.tensor_mul(CX, CX, dx2)
                nc.scalar.activation(CY, CX, AF.Relu, bias=b_half, scale=-1.0)
            pen_s = ps.tile([NROWS, V], F32, tag="sp_s")
            nc.vector.tensor_tensor_scan(
                pen_s, M[:, 0:V], CY, 0.0, OP.mult, OP.add
            )
            # wt = pen_s*rcl (in place)
            nc.gpsimd.tensor_mul(pen_s, pen_s, rcl)

            # ---- validity: q = iota - c_s; valid iff 0 <= q <= W-1,
            # i.e. |q - (W-1)/2| <= (W-1)/2 (iota2 pre-shifted) ----
            q = ps.tile([NROWS, V], F32, tag="e")
            nc.vector.scalar_tensor_tensor(
                q, c_s, -1.0, iota2, OP.mult, OP.add
            )
            v1 = ps.tile([NROWS, V], F32, tag="mx_s")
            if KNOBS["custom_dve"]:
                # wi = w*islast, then wvl = (|q| in range) * wi fused
                nc.gpsimd.tensor_mul(v1, w_t, islast)
                hw = (W - 1) / 2.0 + 0.5
                nc.vector._custom_dve(
                    OP_RANGEW, out=v1, in0=q, in1=v1, s0=hw, s1=-hw
                )
            elif KNOBS["abs_validity"]:
                nc.scalar.activation(q, q, AF.Abs, bias=b_zero)
                nc.gpsimd.tensor_scalar(
                    v1, q, (W - 1) / 2.0, None, OP.is_le
                )
                nc.gpsimd.tensor_mul(v1, v1, w_t)
                nc.gpsimd.tensor_mul(v1, v1, islast)
            else:
                hw = (W - 1) / 2.0
                v2 = ps.tile([NROWS, V], F32, tag="vb")
                nc.gpsimd.tensor_scalar(v1, q, -hw, None, OP.is_ge)
                nc.gpsimd.tensor_scalar(v2, q, hw, None, OP.is_le)
                nc.gpsimd.tensor_mul(v1, v1, v2)
                nc.gpsimd.tensor_mul(v1, v1, w_t)
                nc.gpsimd.tensor_mul(v1, v1, islast)

            # ---- final ----
            # wlx = acc_r + acc_vb (DMA accumulate); wl = relu (in place)
            _acc(acc_r, acc_vb)
            nc.scalar.activation(acc_r, acc_r, AF.Relu, bias=b_zero)
            # f1 = (wt + 1) * wl (into pen_s)
            nc.vector.scalar_tensor_tensor(
                pen_s, pen_s, 1.0, acc_r, OP.add, OP.mult
            )
            # f2 = f1 * wvl, accumulate row-sums
            f2 = pw.tile([NROWS, V], F32, tag="acc_vb")
            acc_j = pw.tile([NROWS, 1], F32, tag="acc_j")
            nc.vector.scalar_tensor_tensor(
                f2, pen_s, 0.0, v1, OP.add, OP.mult, accum_out=acc_j
            )
            nc.vector.tensor_add(acc_total, acc_total, acc_j)

        nc.sync.dma_start(d_out[:, :], acc_total)
    nc.compile()
    return nc


_PROG_CACHE = {}


def _get_program(SW, H):
    key = (SW, H)
    if key not in _PROG_CACHE:
        _PROG_CACHE[key] = build_program(SW, H)
    return _PROG_CACHE[key]


def prepare(pos, pin_dir_x, pin_dir_y, net_weights, pin2net_map, net_mask,
            pin_mask=None):
    """Host-side sharding/padding. Returns (nc, in_maps, meta)."""
    P = int(pin_dir_x.shape[0])
    x = np.ascontiguousarray(np.asarray(pos[:P], dtype=np.float32))
    y = np.ascontiguousarray(np.asarray(pos[P:], dtype=np.float32))
    seg = np.asarray(pin2net_map, dtype=np.int32)
    px = np.asarray(pin_dir_x, dtype=np.float32)
    py = np.asarray(pin_dir_y, dtype=np.float32)
    wm = np.asarray(net_weights, dtype=np.float32) * np.asarray(
        net_mask
    ).astype(np.float32)
    w_pin = wm[seg]

    # max run length -> halo (needs H >= Lmax - 1; +1 margin, mult of 8)
    counts = np.bincount(seg)
    Lmax = int(counts.max()) if counts.size else 1
    H = max(24, -(-(Lmax + 1) // 8) * 8)

    # net-boundary shard splits
    bounds = [0]
    for c in range(1, NCORES):
        tgt = c * P // NCORES
        bounds.append(int(np.searchsorted(seg, seg[tgt], side="left")))
    bounds.append(P)
    maxL = max(bounds[i + 1] - bounds[i] for i in range(NCORES))
    SW = -(-maxL // NROWS)
    SW = -(-SW // 32) * 32  # multiple of 32 (W = SW/4 multiple of 8)
    PAD = NROWS * SW + H + 2

    in_maps = []
    for c in range(NCORES):
        lo, hi = bounds[c], bounds[c + 1]
        L = hi - lo

        def padarr(a, fill, dtype):
            out = np.full(PAD, fill, dtype)
            out[1 : 1 + L] = a[lo:hi]
            return out

        in_maps.append(
            {
                "seg": padarr(seg, -1, np.int32),
                "x": padarr(x, 0.0, np.float32),
                "y": padarr(y, 0.0, np.float32),
                "px": padarr(px, 0.0, np.float32),
                "py": padarr(py, 0.0, np.float32),
                "w": padarr(w_pin, 0.0, np.float32),
            }
        )
    nc = _get_program(SW, H)
    return nc, in_maps, {"SW": SW, "H": H, "PAD": PAD}


def kernel(**inputs):
    nc, in_maps, _ = prepare(**inputs)
    res = run_bass_kernel_spmd(nc, in_maps, list(range(NCORES)))
    total = np.float64(0.0)
    for r in res.results:
        total += np.asarray(r["out"], dtype=np.float64).sum()
    return np.float32(total)


if __name__ == "__main__":
    # tiny self-check with synthetic data
    rng = np.random.default_rng(0)
    Np, Nn = 1 << 14, 1 << 11
    seg = np.sort(rng.integers(0, Nn, Np)).astype(np.int32)
    inputs = dict(
        pos=rng.normal(size=2 * Np).astype(np.float32) * 100,
        pin_dir_x=rng.normal(size=Np).astype(np.float32),
        pin_dir_y=rng.normal(size=Np).astype(np.float32),
        net_weights=rng.random(Nn).astype(np.float32),
        pin2net_map=seg,
        net_mask=np.ones(Nn, bool),
        pin_mask=np.zeros(Np, bool),
    )
    print("result:", kernel(**inputs))
